# revision 9
# baseline (speedup 1.0000x reference)
"""BlazeEar NMS detection kernel for 8 Trainium2 NeuronCores.

Pipeline (SPMD, anchor axis sharded 8 ways):
  per core: load its 512K raw scores -> per-partition top-8 (DVE max/max_index)
  -> exact local threshold at the 33rd-largest candidate (gpsimd kth_largest)
  -> compact surviving (score, global_index) pairs (gpsimd sparse_gather),
     gather each survivor's raw box + anchor row (indirect DMA)
  -> AllGather 32 candidate rows [score|box|anchor] per core
  -> every core sorts the 256-slot pool with 13 max8 rounds (stable, so f32
     score ties resolve by ascending global index, matching jax.lax.top_k)
  -> indirect-gather the sorted top-100 rows, decode boxes, IoU matrix,
     greedy-NMS via Jacobi fixed point, confidence mask -> (100, 5) output.
"""

import sys

sys.path.insert(0, "/opt/trn_rl_repo")

import numpy as np

import concourse.bass as bass
import concourse.bacc as bacc
import concourse.mybir as mybir
from concourse.tile import TileContext

A = 4194304
NCORES = 8
SLAB = A // NCORES          # 524288
P = 128
F = SLAB // P               # 4096
K = 100
NROUNDS = 13                # 13 * 8 = 104 >= 100 extracted values
KPAD = NROUNDS * 8          # 104
LOCAL_K = 31                # kth_largest k_adj -> threshold = 33rd largest local cand
SLOTS = 32                  # compacted candidate slots shipped per core
GLOB = NCORES * SLOTS       # 256
NROW = 9                    # candidate row: [score, rb0..rb3, ax, ay, aw, ah]
NMS_ITERS = 4
INPUT_SIZE_INV = 1.0 / 128.0
CONF = 0.75
IOU = 0.3
NEG = -1e30

f32 = mybir.dt.float32
i32 = mybir.dt.int32
u32 = mybir.dt.uint32
Alu = mybir.AluOpType
Act = mybir.ActivationFunctionType


def _build_program(debug=False):
    nc = bacc.Bacc()

    scores = nc.declare_dram_parameter("scores", [P, F], f32, isOutput=False)
    boxes4 = nc.declare_dram_parameter("boxes4", [SLAB, 4], f32, isOutput=False)
    anchors4 = nc.declare_dram_parameter("anchors4", [SLAB, 4], f32, isOutput=False)
    row_base = nc.declare_dram_parameter("row_base", [P, 1], f32, isOutput=False)
    base_col = nc.declare_dram_parameter("base_col", [P, 1], f32, isOutput=False)
    ut = nc.declare_dram_parameter("ut", [K, K], f32, isOutput=False)
    ramp = nc.declare_dram_parameter("ramp", [16, SLOTS // 16], f32, isOutput=False)
    out = nc.declare_dram_parameter("out", [K, 5], f32, isOutput=True)

    dbg = {}
    if debug:
        for name, shape, dt in [
            ("d_v8", [P, 8], f32), ("d_kth", [1, 2], f32),
            ("d_ccin", [SLOTS, NROW], f32), ("d_ccout", [GLOB, NROW], f32),
            ("d_Va0", [1, GLOB], f32), ("d_SV", [1, KPAD], f32),
            ("d_SI", [1, KPAD], u32), ("d_g9", [K, NROW], f32),
            ("d_K6", [K, 6], f32), ("d_keep", [K, 1], f32),
        ]:
            dbg[name] = nc.declare_dram_parameter(name, shape, dt, isOutput=True)

    # internal DRAM
    gdram_b = nc.dram_tensor("gdram_b", [P * 8], f32)   # Gm bounce (128x8)
    vdram_b = nc.dram_tensor("vdram_b", [P * 8], f32)   # Vm bounce
    g32d = nc.dram_tensor("g32d", [SLOTS], f32)         # slot-ordered indices
    nf_dram = nc.dram_tensor("nf_dram", [1], f32)
    cc_in = nc.dram_tensor("cc_in", [SLOTS, NROW], f32)
    cc_out = nc.dram_tensor("cc_out", [GLOB, NROW], f32, addr_space="Shared")
    si_dram = nc.dram_tensor("si_dram", [KPAD], f32)
    k6_dram = nc.dram_tensor("k6_dram", [6, K], f32)

    with TileContext(nc) as tc:
        with (
            tc.tile_pool(name="big", bufs=1) as bigp,
            tc.tile_pool(name="small", bufs=1) as sp,
            tc.tile_pool(name="psum", bufs=2, space="PSUM") as pp,
        ):
            # ---- stage A: load scores (chunked), per-partition top-8 ----
            S = bigp.tile([P, F], f32)
            NCH = 4
            w = F // NCH
            V32 = sp.tile([P, 8 * NCH], f32)
            for ci in range(NCH):
                nc.sync.dma_start(out=S[:, ci * w:(ci + 1) * w],
                                  in_=scores[:, ci * w:(ci + 1) * w])
                nc.vector.max(out=V32[:, ci * 8:(ci + 1) * 8], in_=S[:, ci * w:(ci + 1) * w])
            V8 = sp.tile([P, 8], f32)
            I8 = sp.tile([P, 8], u32)
            nc.vector.max(out=V8[:], in_=V32[:])
            nc.vector.max_index(out=I8[:], in_max=V8[:], in_values=S[:])
            if debug:
                nc.sync.dma_start(out=dbg["d_v8"][:], in_=V8[:])

            # global candidate indices
            I8f = sp.tile([P, 8], f32)
            nc.vector.tensor_copy(out=I8f[:], in_=I8[:])
            rb = sp.tile([P, 1], f32)
            nc.sync.dma_start(out=rb[:], in_=row_base[:])
            G8 = sp.tile([P, 8], f32)
            nc.vector.tensor_scalar(G8[:], I8f[:], rb[:, 0:1], None, op0=Alu.add)

            # ---- stage B: local threshold + compaction ----
            kth = sp.tile([1, 2], f32)
            nc.gpsimd.kth_largest(kth[:], V8[:], n_per_lane=8, k=64,
                                  quantile=1.0 - (LOCAL_K + 0.5) / (P * 8 - 1))
            if debug:
                nc.sync.dma_start(out=dbg["d_kth"][:], in_=kth[:])
            tb = sp.tile([P, 1], f32)
            nc.gpsimd.partition_broadcast(tb[:], kth[0:1, 1:2])

            m = sp.tile([P, 8], f32)
            nc.vector.tensor_scalar(m[:], V8[:], tb[:, 0:1], None, op0=Alu.is_gt)
            t2 = sp.tile([P, 8], f32)
            nc.vector.tensor_scalar_add(t2[:], m[:], -1.0)
            Gm = sp.tile([P, 8], f32)
            nc.vector.tensor_tensor(Gm[:], G8[:], m[:], op=Alu.mult)
            nc.vector.tensor_tensor(Gm[:], Gm[:], t2[:], op=Alu.add)
            Vm = sp.tile([P, 8], f32)
            nc.vector.tensor_tensor(Vm[:], V8[:], m[:], op=Alu.mult)
            nc.vector.tensor_tensor(Vm[:], Vm[:], t2[:], op=Alu.add)

            nc.sync.dma_start(out=gdram_b[:], in_=Gm[:])
            nc.sync.dma_start(out=vdram_b[:], in_=Vm[:])
            sgin_g = sp.tile([16, 64], f32)
            sgin_v = sp.tile([16, 64], f32)
            # interleaved load: sparse_gather's scan order (f*16+p) must equal
            # the candidate order d[p*8+j] (ascending global index for ties)
            nc.sync.dma_start(out=sgin_g[:], in_=gdram_b[:].rearrange("(b a) -> a b", a=16))
            nc.sync.dma_start(out=sgin_v[:], in_=vdram_b[:].rearrange("(b a) -> a b", a=16))
            sgoG = sp.tile([16, SLOTS // 16], f32)
            sgoV = sp.tile([16, SLOTS // 16], f32)
            nfG = sp.tile([1, 1], u32)
            nfV = sp.tile([1, 1], u32)
            nc.gpsimd.sparse_gather(sgoG[:], sgin_g[:], num_found=nfG[:])
            nc.gpsimd.sparse_gather(sgoV[:], sgin_v[:], num_found=nfV[:])

            # HW sparse_gather leaves garbage past num_found; mask tails to -1
            nff = sp.tile([1, 1], f32)
            nc.vector.tensor_copy(out=nff[:], in_=nfV[:])
            nc.sync.dma_start(out=nf_dram[:], in_=nff[:])
            nfb = sp.tile([16, 1], f32)
            nc.sync.dma_start(out=nfb[:], in_=nf_dram[:].unsqueeze(0).to_broadcast([16, 1]))
            rampt = sp.tile([16, SLOTS // 16], f32)
            nc.sync.dma_start(out=rampt[:], in_=ramp[:])
            msk = sp.tile([16, SLOTS // 16], u32)
            nc.vector.tensor_scalar(msk[:], rampt[:], nfb[:, 0:1], None, op0=Alu.is_lt)
            neg1 = sp.tile([16, SLOTS // 16], f32)
            nc.vector.memset(neg1[:], -1.0)
            sgoVc = sp.tile([16, SLOTS // 16], f32)
            nc.vector.select(sgoVc[:], msk[:], sgoV[:], neg1[:])
            sgoGc = sp.tile([16, SLOTS // 16], f32)
            nc.vector.select(sgoGc[:], msk[:], sgoG[:], neg1[:])

            # slot-ordered global indices -> local row ids -> gather box/anchor
            nc.sync.dma_start(out=g32d[:].rearrange("(b a) -> a b", a=16), in_=sgoGc[:])
            g32 = sp.tile([SLOTS, 1], f32)
            nc.sync.dma_start(out=g32[:], in_=g32d[:].unsqueeze(1))
            bc = sp.tile([P, 1], f32)
            nc.sync.dma_start(out=bc[:], in_=base_col[:])
            li = sp.tile([SLOTS, 1], f32)
            nc.vector.tensor_tensor(li[:], g32[:], bc[0:SLOTS, :], op=Alu.subtract)
            negm = sp.tile([SLOTS, 1], f32)
            nc.vector.tensor_scalar(negm[:], li[:], 0.0, None, op0=Alu.is_lt)
            nc.vector.tensor_scalar_mul(negm[:], negm[:], 8000000.0)
            nc.vector.tensor_tensor(li[:], li[:], negm[:], op=Alu.add)
            lii = sp.tile([SLOTS, 1], i32)
            nc.vector.tensor_copy(out=lii[:], in_=li[:])

            B32 = sp.tile([SLOTS, 4], f32)
            A32 = sp.tile([SLOTS, 4], f32)
            nc.vector.memset(B32[:], 0.0)
            nc.vector.memset(A32[:], 0.0)
            nc.gpsimd.indirect_dma_start(
                out=B32[:], out_offset=None, in_=boxes4[:, :],
                in_offset=bass.IndirectOffsetOnAxis(ap=lii[:, 0:1], axis=0),
                bounds_check=SLAB - 1, oob_is_err=False,
            )
            nc.gpsimd.indirect_dma_start(
                out=A32[:], out_offset=None, in_=anchors4[:, :],
                in_offset=bass.IndirectOffsetOnAxis(ap=lii[:, 0:1], axis=0),
                bounds_check=SLAB - 1, oob_is_err=False,
            )

            # assemble candidate rows [score | box | anchor] in slot order
            nc.sync.dma_start(
                out=cc_in[:, 0].rearrange("(b a) -> a b", a=16), in_=sgoVc[:])
            nc.sync.dma_start(out=cc_in[:, 1:5], in_=B32[:])
            nc.sync.dma_start(out=cc_in[:, 5:9], in_=A32[:])
            if debug:
                nc.sync.dma_start(out=dbg["d_ccin"][:], in_=cc_in[:, :])

            # ---- stage C: AllGather + global sort ----
            nc.gpsimd.collective_compute(
                "AllGather", Alu.bypass,
                replica_groups=[list(range(NCORES))],
                ins=[cc_in[:, :]], outs=[cc_out[:, :]],
            )
            Va = sp.tile([1, GLOB], f32)
            nc.sync.dma_start(out=Va[:], in_=cc_out[:, 0].unsqueeze(0))
            if debug:
                nc.sync.dma_start(out=dbg["d_ccout"][:], in_=cc_out[:, :])
                nc.sync.dma_start(out=dbg["d_Va0"][:], in_=Va[:])

            SV = sp.tile([1, KPAD], f32)
            SI = sp.tile([1, KPAD], u32)
            for r in range(NROUNDS):
                m8 = sp.tile([1, 8], f32, tag="m8")
                i8 = sp.tile([1, 8], u32, tag="i8")
                nc.vector.max(out=m8[:], in_=Va[:])
                nc.vector.max_index(out=i8[:], in_max=m8[:], in_values=Va[:])
                nc.vector.match_replace(out=Va[:], in_to_replace=m8[:],
                                        in_values=Va[:], imm_value=NEG)
                nc.scalar.copy(out=SV[:, r * 8:(r + 1) * 8], in_=m8[:])
                nc.scalar.copy(out=SI[:, r * 8:(r + 1) * 8], in_=i8[:])
            if debug:
                nc.sync.dma_start(out=dbg["d_SV"][:], in_=SV[:])
                nc.sync.dma_start(out=dbg["d_SI"][:], in_=SI[:])

            # gather the winning rows in sorted order
            SIf = sp.tile([1, KPAD], f32)
            nc.vector.tensor_copy(out=SIf[:], in_=SI[:])
            nc.sync.dma_start(out=si_dram[:], in_=SIf[:])
            sic = sp.tile([KPAD, 1], f32)
            nc.sync.dma_start(out=sic[:], in_=si_dram[:].unsqueeze(1))
            sii = sp.tile([KPAD, 1], i32)
            nc.vector.tensor_copy(out=sii[:], in_=sic[:])
            g9 = sp.tile([K, NROW], f32)
            nc.gpsimd.indirect_dma_start(
                out=g9[:], out_offset=None,
                in_=cc_out[:, :],
                in_offset=bass.IndirectOffsetOnAxis(ap=sii[0:K, 0:1], axis=0),
                bounds_check=GLOB - 1, oob_is_err=False,
            )
            if debug:
                nc.sync.dma_start(out=dbg["d_g9"][:], in_=g9[:])

            # ---- stage D: decode in column layout (100 partitions) ----
            rb0, rb1 = g9[:, 1:2], g9[:, 2:3]
            rb2, rb3 = g9[:, 3:4], g9[:, 4:5]
            ax, ay, aw, ah = g9[:, 5:6], g9[:, 6:7], g9[:, 7:8], g9[:, 8:9]

            def tt(o, a, b, op):
                nc.vector.tensor_tensor(o, a, b, op=op)

            K6 = sp.tile([K, 6], f32)   # [y1, x1, y2, x2, area, score]
            xc = sp.tile([K, 1], f32)
            nc.vector.tensor_scalar_mul(xc[:], rb0, INPUT_SIZE_INV)
            tt(xc[:], xc[:], aw, Alu.mult)
            tt(xc[:], xc[:], ax, Alu.add)
            yc = sp.tile([K, 1], f32)
            nc.vector.tensor_scalar_mul(yc[:], rb1, INPUT_SIZE_INV)
            tt(yc[:], yc[:], ah, Alu.mult)
            tt(yc[:], yc[:], ay, Alu.add)
            wh = sp.tile([K, 1], f32)
            nc.vector.tensor_scalar_mul(wh[:], rb2, INPUT_SIZE_INV)
            tt(wh[:], wh[:], aw, Alu.mult)
            nc.vector.tensor_scalar_mul(wh[:], wh[:], 0.5)
            hh = sp.tile([K, 1], f32)
            nc.vector.tensor_scalar_mul(hh[:], rb3, INPUT_SIZE_INV)
            tt(hh[:], hh[:], ah, Alu.mult)
            nc.vector.tensor_scalar_mul(hh[:], hh[:], 0.5)

            ymin0 = sp.tile([K, 1], f32)
            ymax0 = sp.tile([K, 1], f32)
            xmin0 = sp.tile([K, 1], f32)
            xmax0 = sp.tile([K, 1], f32)
            tt(ymin0[:], yc[:], hh[:], Alu.subtract)
            tt(ymax0[:], yc[:], hh[:], Alu.add)
            tt(xmin0[:], xc[:], wh[:], Alu.subtract)
            tt(xmax0[:], xc[:], wh[:], Alu.add)
            tt(K6[:, 0:1], ymin0[:], ymax0[:], Alu.min)
            tt(K6[:, 2:3], ymin0[:], ymax0[:], Alu.max)
            tt(K6[:, 1:2], xmin0[:], xmax0[:], Alu.min)
            tt(K6[:, 3:4], xmin0[:], xmax0[:], Alu.max)
            dxr = sp.tile([K, 1], f32)
            tt(dxr[:], K6[:, 3:4], K6[:, 1:2], Alu.subtract)
            tt(K6[:, 4:5], K6[:, 2:3], K6[:, 0:1], Alu.subtract)
            tt(K6[:, 4:5], K6[:, 4:5], dxr[:], Alu.mult)
            scl = sp.tile([K, 1], f32)
            nc.vector.tensor_scalar_min(scl[:], g9[:, 0:1], 100.0)
            nc.vector.tensor_scalar_max(scl[:], scl[:], -100.0)
            nc.scalar.activation(K6[:, 5:6], scl[:], Act.Sigmoid)
            if debug:
                nc.sync.dma_start(out=dbg["d_K6"][:], in_=K6[:])

            # rows of k6 to DRAM for partition-broadcast loads
            for j in range(5):
                nc.sync.dma_start(out=k6_dram[j, :].unsqueeze(1), in_=K6[:, j:j + 1])

            # ---- stage E: NMS ----
            y1c, x1c, y2c, x2c = K6[:, 0:1], K6[:, 1:2], K6[:, 2:3], K6[:, 3:4]
            areac, scorec = K6[:, 4:5], K6[:, 5:6]
            By1 = bigp.tile([K, K], f32, tag="By1")
            Bx1 = bigp.tile([K, K], f32, tag="Bx1")
            By2 = bigp.tile([K, K], f32, tag="By2")
            Bx2 = bigp.tile([K, K], f32, tag="Bx2")
            Bar = bigp.tile([K, K], f32, tag="Bar")
            for btile, j in [(By1, 0), (Bx1, 1), (By2, 2), (Bx2, 3), (Bar, 4)]:
                nc.sync.dma_start(out=btile[:], in_=k6_dram[j:j + 1, :].to_broadcast([K, K]))

            xx1 = bigp.tile([K, K], f32, tag="xx1")
            nc.vector.tensor_scalar(xx1[:], Bx1[:], x1c, None, op0=Alu.max)
            xx2 = bigp.tile([K, K], f32, tag="xx2")
            nc.vector.tensor_scalar(xx2[:], Bx2[:], x2c, None, op0=Alu.min)
            dx = bigp.tile([K, K], f32, tag="dx")
            tt(dx[:], xx2[:], xx1[:], Alu.subtract)
            nc.vector.tensor_scalar_max(dx[:], dx[:], 0.0)
            yy1 = bigp.tile([K, K], f32, tag="yy1")
            nc.vector.tensor_scalar(yy1[:], By1[:], y1c, None, op0=Alu.max)
            yy2 = bigp.tile([K, K], f32, tag="yy2")
            nc.vector.tensor_scalar(yy2[:], By2[:], y2c, None, op0=Alu.min)
            dy = bigp.tile([K, K], f32, tag="dy")
            tt(dy[:], yy2[:], yy1[:], Alu.subtract)
            nc.vector.tensor_scalar_max(dy[:], dy[:], 0.0)
            inter = bigp.tile([K, K], f32, tag="inter")
            tt(inter[:], dx[:], dy[:], Alu.mult)
            un = bigp.tile([K, K], f32, tag="un")
            nc.vector.tensor_scalar(un[:], Bar[:], areac, None, op0=Alu.add)
            tt(un[:], un[:], inter[:], Alu.subtract)
            nc.vector.tensor_scalar_max(un[:], un[:], 1e-9)
            nc.vector.tensor_scalar_mul(un[:], un[:], IOU)
            M = bigp.tile([K, K], f32, tag="M")
            tt(M[:], inter[:], un[:], Alu.is_gt)
            UT = bigp.tile([K, K], f32, tag="UT")
            nc.sync.dma_start(out=UT[:], in_=ut[:, :])
            tt(M[:], M[:], UT[:], Alu.mult)

            keep = sp.tile([K, 1], f32)
            nc.vector.memset(keep[:], 1.0)
            for _ in range(NMS_ITERS):
                kv = pp.tile([K, 1], f32, tag="kv")
                nc.tensor.matmul(kv[:], M[:], keep[:])
                nc.vector.tensor_scalar(keep[:], kv[:], 0.5, None, op0=Alu.is_lt)
            cm = sp.tile([K, 1], f32)
            nc.vector.tensor_scalar(cm[:], scorec, CONF, None, op0=Alu.is_ge)
            tt(keep[:], keep[:], cm[:], Alu.mult)
            if debug:
                nc.sync.dma_start(out=dbg["d_keep"][:], in_=keep[:])

            O = sp.tile([K, 5], f32)
            nc.vector.tensor_scalar(O[:, 0:4], K6[:, 0:4], keep[:, 0:1], None, op0=Alu.mult)
            nc.vector.tensor_scalar(O[:, 4:5], scorec, keep[:, 0:1], None, op0=Alu.mult)
            nc.sync.dma_start(out=out[:], in_=O[:])

    nc.finalize()
    return nc


_NC_CACHE = None


def _get_nc():
    global _NC_CACHE
    if _NC_CACHE is None:
        _NC_CACHE = _build_program()
    return _NC_CACHE


def _make_in_maps(raw_boxes, raw_scores, anchors):
    raw_boxes = np.asarray(raw_boxes)
    raw_scores = np.asarray(raw_scores)
    anchors = np.asarray(anchors)
    ut_np = np.triu(np.ones((K, K), np.float32), k=1)
    ramp_np = np.arange(SLOTS, dtype=np.float32).reshape(SLOTS // 16, 16).T.copy()
    in_maps = []
    for c in range(NCORES):
        s = slice(c * SLAB, (c + 1) * SLAB)
        in_maps.append({
            "scores": np.ascontiguousarray(raw_scores[0, s, 0].reshape(P, F)),
            "boxes4": np.ascontiguousarray(raw_boxes[0, s, 0:4]),
            "anchors4": np.ascontiguousarray(anchors[s]),
            "row_base": (c * SLAB + np.arange(P, dtype=np.float32) * F).reshape(P, 1),
            "base_col": np.full((P, 1), c * SLAB, np.float32),
            "ut": ut_np,
            "ramp": ramp_np,
        })
    return in_maps


def kernel(raw_boxes, raw_scores, anchors):
    from concourse.bass_utils import run_bass_kernel_spmd
    nc = _get_nc()
    in_maps = _make_in_maps(raw_boxes, raw_scores, anchors)
    res = run_bass_kernel_spmd(nc, in_maps, list(range(NCORES)))
    return np.asarray(res.results[0]["out"], dtype=np.float32)
